# revision 15
# baseline (speedup 1.0000x reference)
"""Trainium2 Bass kernel for nn_FeatureFunMax (segment_reduce).

Math (per batch b, all fp32 in/out):
  whole[t,l]  = sum_h max(x[t,h], x[t-l,h]) * whole_w[l,h]    (x[t-l]=0 for t<l)
  end[t]      = sum_h x[t,h] * end_w[h]
  out[t,l]    = whole[t,l] + end[t] + length_bias[l]
  relay[t]    = sum_h max(x[t,h], x[t+16,h]) * relay_w[h] + relay_b

Sharding: batch-parallel, one batch per NeuronCore (B=8, n_cores=8). No halo
needed since each core owns a full [T,H] batch.

Per-core plan (layout [h partitions, t free] so time-shifts are AP offsets):
  1. gpsimd cast-DMA x fp32 [T,H] -> DRAM scratch bf16
  2. HWDGE xbar dma-transpose -> SBUF X_ext [128, 2(kt), 16+T+16] bf16 with
     zero halo columns (so t<0 / t>=T reads give max(x,0) for free)
  3. X1 = X_ext shifted 1 col (ACT copies) so odd-lag TT operands stay
     4B-aligned and keep DVE's bf16 2x mode
  4. 16 shifted tensor_max streams (15 lags + relay) on DVE/GPSIMD
  5. PE col-tiled (128x32) m=1 matvecs: psum[strip] += w_l . Ml (2 k-tiles)
     plus end_w . x folded in as 2 extra accumulating matmuls per lag stream
  6. ACT evacuates psum strips -> SBUF with fused per-partition bias add
     (length_bias / relay_b), HWDGE DMA out res [17, T] fp32
Host: out = res[0:16].T, relay = res[16] per batch (pure unshard/transpose).
"""

import numpy as np
import ml_dtypes

import concourse.bacc as bacc
import concourse.tile as tile
from concourse import mybir
import concourse.bass_utils as bass_utils

B, T, H, L = 8, 2048, 256, 16
CH = 512                      # chunk of t (one PSUM bank of fp32)
NCH = T // CH                 # 4
W = 16 + T + 16               # padded width of X_ext (2080)
F32 = mybir.dt.float32
BF16 = mybir.dt.bfloat16

# stream ids: 0 = C0 (l=0, plain matvec), 1..15 = lag l, 16 = relay
GROUPS = [[0, 1, 2, 3, 4], [5, 6, 7, 8], [9, 10, 11, 12], [13, 14, 15, 16]]
GPS_SIDS = set()              # gpsimd TT(bf16) fails the Pool ISA check

# PE weights: walrus requires matmul PSUM outputs to start at a quadrant base
# (0/32/64/96), so each m=1 matvec is widened to a one-hot m=32 matmul: the
# lhsT block [128, 32] holds the weight vector in the stream's column within
# its group, zeros elsewhere. All streams of a group accumulate into the same
# [32, CH] PSUM block; the zero columns contribute nothing.
# Block table (host-packed into one [128, NBLK*32] tensor):
#   block index = (sid * 2 + kind) * 2 + kt, kind 0 = main w, kind 1 = end_w
NBLK = 17 * 2 * 2

_CACHED = {}


def _build():
    nc = bacc.Bacc("TRN2", target_bir_lowering=False, debug=False, num_devices=8)
    x = nc.dram_tensor("x", [T, H], F32, kind="ExternalInput").ap()
    wblk = nc.dram_tensor("wblk", [128, NBLK * 32], BF16, kind="ExternalInput").ap()
    biasv = nc.dram_tensor("biasv", [128, 1], F32, kind="ExternalInput").ap()
    res = nc.dram_tensor("res", [17, T], F32, kind="ExternalOutput").ap()
    scratch = nc.dram_tensor("scratch", [T, H], BF16, kind="Internal").ap()

    with tile.TileContext(nc) as tc:
        with (
            tc.tile_pool(name="singles", bufs=1) as singles,
            tc.tile_pool(name="ml", bufs=20) as mlpool,
            tc.tile_pool(name="psum", bufs=4, space="PSUM") as psums,
        ):
            xe = singles.tile([128, 2, W], BF16)    # [h-part, ktile, 16+t+16]
            x1 = singles.tile([128, 2, W], BF16)    # xe shifted left 1 col
            ressb = singles.tile([128, T], F32)
            wsb = singles.tile([128, NBLK, 32], BF16)
            bsb = singles.tile([128, 1], F32)

            nc.sync.dma_start(
                out=wsb, in_=wblk.rearrange("p (b c) -> p b c", c=32)
            )
            nc.sync.dma_start(out=bsb, in_=biasv)

            # zero halo columns (left 16 incl. X1's shifted view, right 16)
            nc.vector.memset(xe[:, :, 0:16], 0.0)
            nc.vector.memset(xe[:, :, 16 + T: W], 0.0)
            nc.vector.memset(x1[:, :, 0:16], 0.0)
            nc.vector.memset(x1[:, :, 15 + T: W], 0.0)

            psum_c = []
            for c in range(NCH):
                psum_c.append(
                    psums.tile([128, CH], F32, name=f"psum{c}", tag="psum")
                )

            # input pipeline for ALL chunks first, so every later read
            # (incl. relay's right halo into chunk c+1) is a true RAW dep
            for c in range(NCH):
                lo, hi = c * CH, (c + 1) * CH
                # 1) cast fp32 -> bf16 into DRAM scratch (SWDGE cast DMA)
                nc.gpsimd.dma_start(out=scratch[lo:hi, :], in_=x[lo:hi, :])
                # 2) transpose each 128-wide h slab into X_ext columns
                for kt in range(2):
                    nc.sync.dma_start(
                        out=xe[:, kt, 16 + lo: 16 + hi],
                        in_=scratch[lo:hi, kt * 128:(kt + 1) * 128],
                        transpose=True,
                    )
                # 3) X1 = X_ext << 1 column (keeps odd-lag reads 4B aligned)
                nc.scalar.copy(
                    out=x1[:, :, 15 + lo: 15 + hi],
                    in_=xe[:, :, 16 + lo: 16 + hi],
                )

            ml_tiles = {}
            for c in range(NCH):
                lo, hi = c * CH, (c + 1) * CH
                # 4) shifted maxes for this chunk
                for sid in range(1, 17):
                    eng = nc.gpsimd if sid in GPS_SIDS else nc.vector
                    ml = mlpool.tile(
                        [128, 2, CH], BF16, name=f"ml{sid}_{c}", tag="ml"
                    )
                    if sid == 16:  # relay: max(x[t], x[t+16])
                        in1 = xe[:, :, 32 + lo: 32 + hi]
                    elif sid % 2 == 0:  # even lag
                        in1 = xe[:, :, 16 + lo - sid: 16 + hi - sid]
                    else:  # odd lag via the 1-shifted copy
                        in1 = x1[:, :, 15 + lo - sid: 15 + hi - sid]
                    eng.tensor_max(ml, xe[:, :, 16 + lo: 16 + hi], in1)
                    ml_tiles[(sid, c)] = ml

                # 5) PE matvec accumulation into per-group psum blocks
                xch = [xe[:, kt, 16 + lo: 16 + hi] for kt in range(2)]
                for g, sids in enumerate(GROUPS):
                    outp = psum_c[c][32 * g: 32 * g + 32, :]
                    mms = []  # (blk, rhs)
                    for sid in sids:
                        if sid == 0 or sid == 16:
                            kinds = [0]
                        else:
                            kinds = [0, 1]
                        for kind in kinds:
                            for kt in range(2):
                                blk = (sid * 2 + kind) * 2 + kt
                                if sid != 0 and kind == 0:
                                    rhs = ml_tiles[(sid, c)][:, kt, :]
                                else:
                                    rhs = xch[kt]
                                mms.append((blk, rhs))
                    n = len(mms)
                    for i, (blk, rhs) in enumerate(mms):
                        nc.tensor.matmul(
                            out=outp,
                            lhsT=wsb[:, blk, :],
                            rhs=rhs,
                            start=(i == 0),
                            stop=(i == n - 1),
                            tile_position=(0, 32 * g),
                            skip_group_check=True,
                        )

                # 6) evacuate strips + bias add (ACT), then DMA out
                row0 = 0
                for g, sids in enumerate(GROUPS):
                    cnt = len(sids)
                    p0 = 32 * g
                    nc.scalar.activation(
                        out=ressb[p0:p0 + cnt, lo:hi],
                        in_=psum_c[c][p0:p0 + cnt, :],
                        func=mybir.ActivationFunctionType.Identity,
                        bias=bsb[p0:p0 + cnt],
                        scale=1.0,
                    )
                    nc.sync.dma_start(
                        out=res[row0:row0 + cnt, lo:hi],
                        in_=ressb[p0:p0 + cnt, lo:hi],
                    )
                    row0 += cnt
    nc.finalize()
    return nc


def _get_nc():
    if "nc" not in _CACHED:
        _CACHED["nc"] = _build()
    return _CACHED["nc"]


def _prep_consts(end_w, whole_w, relay_w, relay_b, length_bias):
    sid_col = {}
    for g, sids in enumerate(GROUPS):
        for idx, sid in enumerate(sids):
            sid_col[sid] = idx

    wblk = np.zeros((128, NBLK, 32), dtype=np.float32)
    for sid in range(17):
        col = sid_col[sid]
        if sid == 0:
            mains = whole_w[0] + end_w
        elif sid == 16:
            mains = relay_w
        else:
            mains = whole_w[sid]
        for kt in range(2):
            sl = slice(kt * 128, (kt + 1) * 128)
            wblk[:, (sid * 2 + 0) * 2 + kt, col] = mains[sl]
            if 1 <= sid <= 15:
                wblk[:, (sid * 2 + 1) * 2 + kt, col] = end_w[sl]
    wblk = wblk.reshape(128, NBLK * 32).astype(ml_dtypes.bfloat16)

    biasv = np.zeros((128, 1), dtype=np.float32)
    sid2bias = {}
    for l in range(16):
        sid2bias[l] = float(length_bias[l])
    sid2bias[16] = float(relay_b)
    for g, sids in enumerate(GROUPS):
        for idx, sid in enumerate(sids):
            biasv[32 * g + idx, 0] = sid2bias[sid]
    return wblk, biasv


def kernel(logits, end_w, whole_w, relay_w, relay_b, length_bias):
    logits = np.asarray(logits, dtype=np.float32)
    end_w = np.asarray(end_w, dtype=np.float32)
    whole_w = np.asarray(whole_w, dtype=np.float32)
    relay_w = np.asarray(relay_w, dtype=np.float32)
    relay_b = np.asarray(relay_b, dtype=np.float32)
    length_bias = np.asarray(length_bias, dtype=np.float32)

    nc = _get_nc()
    wblk, biasv = _prep_consts(end_w, whole_w, relay_w, relay_b, length_bias)
    in_maps = [
        {"x": np.ascontiguousarray(logits[b]), "wblk": wblk, "biasv": biasv}
        for b in range(B)
    ]
    r = bass_utils.run_bass_kernel_spmd(nc, in_maps, core_ids=list(range(B)))
    out = np.empty((B, T, L), dtype=np.float32)
    relay = np.empty((B, T), dtype=np.float32)
    for b in range(B):
        resb = r.results[b]["res"]
        out[b] = resb[0:16].T
        relay[b] = resb[16]
    return out, relay


# revision 21
# speedup vs baseline: 2.9225x; 2.9225x over previous
"""Trainium2 Bass kernel for nn_FeatureFunMax (segment_reduce).

Math (per batch b, all fp32 in/out):
  whole[t,l]  = sum_h max(x[t,h], x[t-l,h]) * whole_w[l,h]    (x[t-l]=0 for t<l)
  end[t]      = sum_h x[t,h] * end_w[h]
  out[t,l]    = whole[t,l] + end[t] + length_bias[l]
  relay[t]    = sum_h max(x[t,h], x[t+16,h]) * relay_w[h] + relay_b

Sharding: batch-parallel, one batch per NeuronCore (B=8, n_cores=8). No halo
needed since each core owns a full [T,H] batch.

Per-core plan (layout [h partitions, t free] so time-shifts are AP offsets):
  1. gpsimd cast-DMA x fp32 [T,H] -> DRAM scratch bf16
  2. HWDGE xbar dma-transpose -> SBUF X_ext [128, 2(kt), 16+T+16] bf16 with
     zero halo columns (so t<0 / t>=T reads give max(x,0) for free)
  3. X1 = X_ext shifted 1 col (ACT copies) so odd-lag TT operands stay
     4B-aligned and keep DVE's bf16 2x mode
  4. 16 shifted tensor_max streams (15 lags + relay) on DVE/GPSIMD
  5. PE col-tiled (128x32) m=1 matvecs: psum[strip] += w_l . Ml (2 k-tiles)
     plus end_w . x folded in as 2 extra accumulating matmuls per lag stream
  6. ACT evacuates psum strips -> SBUF with fused per-partition bias add
     (length_bias / relay_b), HWDGE DMA out res [17, T] fp32
Host: out = res[0:16].T, relay = res[16] per batch (pure unshard/transpose).
"""

import numpy as np
import ml_dtypes

import concourse.bass as bass
import concourse.bacc as bacc
import concourse.tile as tile
from concourse import mybir
import concourse.bass_utils as bass_utils

B, T, H, L = 8, 2048, 256, 16
CH = 512                      # chunk of t (one PSUM bank of fp32)
NCH = T // CH                 # 4
W = 16 + T + 16               # padded width of X_ext (2080)
F32 = mybir.dt.float32
BF16 = mybir.dt.bfloat16

# stream ids: 0 = C0 (l=0, matvec with w0+end_w), 1..15 = lag l, 16 = relay,
# 17 = END (x . end_w); END is added onto the lag rows post-evac via tiny
# SWDGE broadcast-accumulate DMAs instead of per-lag extra matmuls.
GROUPS = [[0, 1, 2, 3, 4], [5, 6, 7, 8, 17], [9, 10, 11, 12], [13, 14, 15, 16]]
END_PART = 36                 # psum/sbuf partition of the END stream
GPS_SIDS = set()              # gpsimd TT(bf16) fails the Pool ISA check

# PE weights: walrus requires matmul PSUM outputs to start at a quadrant base
# (0/32/64/96), so each m=1 matvec is widened to a one-hot m=32 matmul: the
# lhsT block [128, 32] holds the weight vector in the stream's column within
# its group, zeros elsewhere. All streams of a group accumulate into the same
# [32, CH] PSUM block; the zero columns contribute nothing.
# Block table (host-packed into one [128, NBLK*32] tensor):
#   block index = sid * 2 + kt
NBLK = 18 * 2

_CACHED = {}


def _build():
    nc = bacc.Bacc("TRN2", target_bir_lowering=False, debug=False, num_devices=8)
    x = nc.dram_tensor("x", [T, H], F32, kind="ExternalInput").ap()
    wblk = nc.dram_tensor("wblk", [128, NBLK * 32], BF16, kind="ExternalInput").ap()
    biasv = nc.dram_tensor("biasv", [128, 1], F32, kind="ExternalInput").ap()
    res = nc.dram_tensor("res", [17, T], F32, kind="ExternalOutput").ap()
    scratch = nc.dram_tensor("scratch", [T, H], BF16, kind="Internal").ap()

    with tile.TileContext(nc) as tc:
        with (
            tc.tile_pool(name="singles", bufs=1) as singles,
            tc.tile_pool(name="ml", bufs=20) as mlpool,
            tc.tile_pool(name="psum", bufs=4, space="PSUM") as psums,
        ):
            xe = singles.tile([128, 2, W], BF16)    # [h-part, ktile, 16+t+16]
            x1 = singles.tile([128, 2, W], BF16)    # xe shifted left 1 col
            ressb = singles.tile([128, T], F32)
            wsb = singles.tile([128, NBLK, 32], BF16)
            bsb = singles.tile([128, 1], F32)

            nc.sync.dma_start(
                out=wsb, in_=wblk.rearrange("p (b c) -> p b c", c=32)
            )
            nc.sync.dma_start(out=bsb, in_=biasv)

            # zero halo columns (left 16 incl. X1's shifted view, right 16)
            nc.vector.memset(xe[:, :, 0:16], 0.0)
            nc.vector.memset(xe[:, :, 16 + T: W], 0.0)
            nc.vector.memset(x1[:, :, 0:16], 0.0)
            nc.vector.memset(x1[:, :, 15 + T: W], 0.0)

            psum_c = []
            for c in range(NCH):
                psum_c.append(
                    psums.tile([128, CH], F32, name=f"psum{c}", tag="psum")
                )

            # input pipeline for ALL chunks first, so every later read
            # (incl. relay's right halo into chunk c+1) is a true RAW dep
            for c in range(NCH):
                lo, hi = c * CH, (c + 1) * CH
                # 1) cast fp32 -> bf16 into DRAM scratch (SWDGE cast DMA)
                nc.gpsimd.dma_start(out=scratch[lo:hi, :], in_=x[lo:hi, :])
                # 2) transpose each 128-wide h slab into X_ext columns
                for kt in range(2):
                    nc.sync.dma_start(
                        out=xe[:, kt, 16 + lo: 16 + hi],
                        in_=scratch[lo:hi, kt * 128:(kt + 1) * 128],
                        transpose=True,
                    )
                # 3) X1 = X_ext << 1 column (keeps odd-lag reads 4B aligned)
                nc.scalar.copy(
                    out=x1[:, :, 15 + lo: 15 + hi],
                    in_=xe[:, :, 16 + lo: 16 + hi],
                )

            ml_tiles = {}
            for c in range(NCH):
                lo, hi = c * CH, (c + 1) * CH
                # 4) shifted maxes for this chunk
                for sid in range(1, 17):
                    eng = nc.gpsimd if sid in GPS_SIDS else nc.vector
                    ml = mlpool.tile(
                        [128, 2, CH], BF16, name=f"ml{sid}_{c}", tag="ml"
                    )
                    if sid == 16:  # relay: max(x[t], x[t+16])
                        in1 = xe[:, :, 32 + lo: 32 + hi]
                    elif sid % 2 == 0:  # even lag
                        in1 = xe[:, :, 16 + lo - sid: 16 + hi - sid]
                    else:  # odd lag via the 1-shifted copy
                        in1 = x1[:, :, 15 + lo - sid: 15 + hi - sid]
                    eng.tensor_max(ml, xe[:, :, 16 + lo: 16 + hi], in1)
                    ml_tiles[(sid, c)] = ml

                # 5) PE matvec accumulation into per-group psum blocks
                xch = [xe[:, kt, 16 + lo: 16 + hi] for kt in range(2)]
                for g, sids in enumerate(GROUPS):
                    outp = psum_c[c][32 * g: 32 * g + 32, :]
                    mms = []  # (blk, rhs)
                    for sid in sids:
                        for kt in range(2):
                            blk = sid * 2 + kt
                            if 1 <= sid <= 16:
                                rhs = ml_tiles[(sid, c)][:, kt, :]
                            else:  # C0 / END read x directly
                                rhs = xch[kt]
                            mms.append((blk, rhs))
                    n = len(mms)
                    for i, (blk, rhs) in enumerate(mms):
                        nc.tensor.matmul(
                            out=outp,
                            lhsT=wsb[:, blk, :],
                            rhs=rhs,
                            start=(i == 0),
                            stop=(i == n - 1),
                            tile_position=(0, 32 * g),
                            skip_group_check=True,
                        )

                # 6) evacuate strips + bias add (ACT)
                for g, sids in enumerate(GROUPS):
                    cnt = len(sids)
                    p0 = 32 * g
                    nc.scalar.activation(
                        out=ressb[p0:p0 + cnt, lo:hi],
                        in_=psum_c[c][p0:p0 + cnt, :],
                        func=mybir.ActivationFunctionType.Identity,
                        bias=bsb[p0:p0 + cnt],
                        scale=1.0,
                    )
                # 7) DMA out
                row0 = 0
                for g, sids in enumerate(GROUPS):
                    cnt = len(sids) if g != 1 else 4  # END row stays on-chip
                    p0 = 32 * g
                    nc.sync.dma_start(
                        out=res[row0:row0 + cnt, lo:hi],
                        in_=ressb[p0:p0 + cnt, lo:hi],
                    )
                    row0 += cnt
                # 8) accumulate end scores onto the (contiguous) lag rows of
                # res in DRAM: one SWDGE CCE-add DMA, END row free-broadcast
                end_src = ressb[END_PART:END_PART + 1, lo:hi]
                end_bcast = bass.AP(
                    tensor=end_src.tensor,
                    offset=end_src.offset,
                    ap=[end_src.ap[0], [0, 15], end_src.ap[1]],
                )
                nc.gpsimd.dma_start(
                    out=res[1:16, lo:hi],
                    in_=end_bcast,
                    accum_op=mybir.AluOpType.add,
                )
    nc.finalize()
    return nc


def _get_nc():
    if "nc" not in _CACHED:
        _CACHED["nc"] = _build()
    return _CACHED["nc"]


def _prep_consts(end_w, whole_w, relay_w, relay_b, length_bias):
    sid_col = {}
    for g, sids in enumerate(GROUPS):
        for idx, sid in enumerate(sids):
            sid_col[sid] = idx

    wblk = np.zeros((128, NBLK, 32), dtype=np.float32)
    for sid in range(18):
        col = sid_col[sid]
        if sid == 0:
            mains = whole_w[0] + end_w
        elif sid == 16:
            mains = relay_w
        elif sid == 17:
            mains = end_w
        else:
            mains = whole_w[sid]
        for kt in range(2):
            sl = slice(kt * 128, (kt + 1) * 128)
            wblk[:, sid * 2 + kt, col] = mains[sl]
    wblk = wblk.reshape(128, NBLK * 32).astype(ml_dtypes.bfloat16)

    biasv = np.zeros((128, 1), dtype=np.float32)
    sid2bias = {}
    for l in range(16):
        sid2bias[l] = float(length_bias[l])
    sid2bias[16] = float(relay_b)
    sid2bias[17] = 0.0
    for g, sids in enumerate(GROUPS):
        for idx, sid in enumerate(sids):
            biasv[32 * g + idx, 0] = sid2bias[sid]
    return wblk, biasv


def kernel(logits, end_w, whole_w, relay_w, relay_b, length_bias):
    logits = np.asarray(logits, dtype=np.float32)
    end_w = np.asarray(end_w, dtype=np.float32)
    whole_w = np.asarray(whole_w, dtype=np.float32)
    relay_w = np.asarray(relay_w, dtype=np.float32)
    relay_b = np.asarray(relay_b, dtype=np.float32)
    length_bias = np.asarray(length_bias, dtype=np.float32)

    nc = _get_nc()
    wblk, biasv = _prep_consts(end_w, whole_w, relay_w, relay_b, length_bias)
    in_maps = [
        {"x": np.ascontiguousarray(logits[b]), "wblk": wblk, "biasv": biasv}
        for b in range(B)
    ]
    r = bass_utils.run_bass_kernel_spmd(nc, in_maps, core_ids=list(range(B)))
    out = np.empty((B, T, L), dtype=np.float32)
    relay = np.empty((B, T), dtype=np.float32)
    for b in range(B):
        resb = r.results[b]["res"]
        out[b] = resb[0:16].T
        relay[b] = resb[16]
    return out, relay
